# revision 1
# baseline (speedup 1.0000x reference)
"""Distributed GAT layer on 8 Trainium2 NeuronCores (Bass/Tile).

Strategy (dst-sharded, degree-aligned ELL):
  * fc projection replicated on every core: h = feat @ W.T plus folded
    attention reductions el = feat @ AL, er = feat @ AR computed in the same
    matmul; rows written to a DRAM table [h bf16 | el hi/lo | er hi/lo | pad]
    of 768 B per node.
  * dst nodes globally sorted by (lo-degree, hi-degree), packed into blocks
    of 128, blocks dealt round-robin to the 8 cores so every core runs the
    identical instruction stream on identically-shaped slot arrays.
  * per block, incoming edges live in an ELL slot array [128 dst x D cols];
    src rows are fetched with dma_gather (two streams, since gather indices
    are int16: table rows < 32768 and the rest), scores computed on DVE/ACT
    (exp without max-subtraction -- magnitudes are bounded), softmax denom
    via strided reduce, weighted aggregation via identity-matmul PSUM
    accumulation, then scale + bias.
"""

import math
from contextlib import ExitStack
from dataclasses import dataclass, field

import ml_dtypes
import numpy as np

import concourse.bass as bass
import concourse.tile as tile
from concourse import bacc, mybir
from concourse.bass_utils import run_bass_kernel_spmd

P = 128
NCORES = 8
H, F = 4, 64
HF = H * F            # 256
IN_FEATS = 256
ROW = 384             # bf16 elements per table row stride (768 B)
ROWU = 272            # payload elements actually gathered (544 B)
H_OFF = 0             # [0:256]   h bf16
EHI_OFF = 256         # [256:260] el hi, [260:264] er hi
ELO_OFF = 264         # [264:268] el lo, [268:272] er lo
NEG = 0.2
HI_BASE = 32768
NEG_MASK = -1.0e30
BF16 = ml_dtypes.bfloat16
TRACE = False
LAST_RESULTS = None


@dataclass
class Cfg:
    n: int
    npad: int
    nt: int                      # fc tiles = npad // 128
    nbc: int                     # block-positions per core
    groups: list = field(default_factory=list)   # (bp0, g, dlo, dhi, base_lo, base_hi)
    slo: int = 0                 # total lo slot-columns per core
    shi: int = 0
    hi_base: int = HI_BASE


def _emit_gather(nc, out_ap, in_ap, idxs_ap, num_idxs, queue_num=0,
                 elem_size=None, elem_step=None):
    """dma_gather with payload < row stride (bass asserts %256 on payload,
    but only the stride is 256B-quantized in the descriptor format --
    verified on HW)."""
    import concourse.ap_utils as ap_utils
    from concourse.bass import MemorySpace
    eng = nc.gpsimd
    elem_size = ROWU if elem_size is None else elem_size
    elem_step = ROW if elem_step is None else elem_step
    assert idxs_ap.dtype == mybir.dt.int16
    assert in_ap.dtype == out_ap.dtype
    assert in_ap.space == MemorySpace.DRAM
    assert idxs_ap.space == MemorySpace.SBUF and out_ap.space == MemorySpace.SBUF
    assert ap_utils.ap_is_contiguous(out_ap.ap[1:])
    assert ap_utils.ap_is_contiguous(idxs_ap.ap[1:])
    assert in_ap.ap[-1][1] == elem_size == out_ap.ap[-1][1]
    assert out_ap.ap[0][1] * out_ap.ap[1][1] == num_idxs
    assert in_ap.ap[0][0] == elem_step
    stride_bytes = elem_step * mybir.dt.size(in_ap.dtype)
    _in_ap = eng.lower_ap_dma(in_ap, for_custom_bir_dma=True)
    _idxs_ap = eng.lower_ap(idxs_ap)
    _out_ap = eng.lower_ap(out_ap)
    return eng.add_instruction(
        mybir.InstDMAGatherAnt(
            name=eng.bass.get_next_instruction_name(),
            ins=[*_in_ap, _idxs_ap, eng.lower_val_access(eng.to_reg(num_idxs))],
            outs=[_out_ap],
            transpose=False, num_idxs=num_idxs, elem_size=elem_size,
            stride_bytes_256=stride_bytes // 256, gen_mode=0,
            single_packet=False, queue_num=queue_num,
            sbuf_tokens_per_rank=0, sbuf_free_dim_per_rank=0,
            sbuf_free_dim_pad_per_rank=0, sbuf_byte_offset=0,
        )
    )


def build_cfg(deg_lo_blk, deg_hi_blk, n, npad, group_max=4):
    """deg_*_blk: [NB] per-block max degree arrays (global block order)."""
    nb = npad // P
    nbc = (nb + NCORES - 1) // NCORES
    # per-position max over the cores' blocks at that position
    dlo_p = np.ones(nbc, np.int64)
    dhi_p = np.ones(nbc, np.int64)
    np.maximum.at(dlo_p, np.arange(nb) // NCORES, deg_lo_blk)
    np.maximum.at(dhi_p, np.arange(nb) // NCORES, deg_hi_blk)

    cfg = Cfg(n=n, npad=npad, nt=npad // P, nbc=nbc)
    base_lo = base_hi = 0
    bp = 0
    budget = 38 * 1024  # bytes/partition for one gather buffer set
    while bp < nbc:
        g = 1
        while (
            bp + g < nbc
            and g < group_max
            and (g + 1)
            * (max(dlo_p[bp : bp + g + 1]) + max(dhi_p[bp : bp + g + 1]))
            * (2 * ROWU)
            <= budget
        ):
            g += 1
        dlo = int(max(dlo_p[bp : bp + g]))
        dhi = int(max(dhi_p[bp : bp + g]))
        cfg.groups.append((bp, g, dlo, dhi, base_lo, base_hi))
        base_lo += g * dlo
        base_hi += g * dhi
        bp += g
    cfg.slo = base_lo
    cfg.shi = base_hi
    return cfg


def host_prep(feat, W, attn_l, attn_r, bias, src, dst, n, hi_base=HI_BASE):
    """Returns (cfg, shared_map, per_core_maps, order) ."""
    e = src.shape[0]
    npad = ((n + P * NCORES - 1) // (P * NCORES)) * P * NCORES  # multiple of 1024
    nb = npad // P

    src = np.asarray(src, np.int64)
    dst = np.asarray(dst, np.int64)
    cls_e = src >= hi_base                       # per-edge: hi stream?
    deg_lo = np.bincount(dst[~cls_e], minlength=n)
    deg_hi = np.bincount(dst[cls_e], minlength=n)

    order = np.lexsort((deg_hi, deg_lo))          # rank -> node
    rank_of = np.empty(n, np.int64)
    rank_of[order] = np.arange(n)

    # block stats in global block order; re-sort blocks by their degree
    # profile so blocks dealt to the same position pad alike
    dlo_r = np.zeros(npad, np.int64)
    dhi_r = np.zeros(npad, np.int64)
    dlo_r[: n] = deg_lo[order]
    dhi_r[: n] = deg_hi[order]
    blk_max_lo = dlo_r.reshape(nb, P).max(1)
    blk_max_hi = dhi_r.reshape(nb, P).max(1)
    border = np.lexsort((blk_max_hi, blk_max_lo, blk_max_hi // 2))  # q -> j
    inv_border = np.empty(nb, np.int64)
    inv_border[border] = np.arange(nb)
    cfg = build_cfg(blk_max_lo[border], blk_max_hi[border], n, npad)
    cfg.hi_base = hi_base

    # per-bp column base within the core-wide slot arrays
    colbase_lo = np.zeros(cfg.nbc, np.int64)
    colbase_hi = np.zeros(cfg.nbc, np.int64)
    dlo_of_bp = np.zeros(cfg.nbc, np.int64)
    dhi_of_bp = np.zeros(cfg.nbc, np.int64)
    for bp0, g, dlo, dhi, b_lo, b_hi in cfg.groups:
        for gi in range(g):
            colbase_lo[bp0 + gi] = b_lo + gi * dlo
            colbase_hi[bp0 + gi] = b_hi + gi * dhi
            dlo_of_bp[bp0 + gi] = dlo
            dhi_of_bp[bp0 + gi] = dhi

    # order edges by (dst, class, non-self) so the self edge heads its class run
    self_e = src == dst
    es = np.lexsort((~self_e, cls_e, dst))
    ds, ss, cs = dst[es], src[es], cls_e[es]
    run = ds * 2 + cs
    first = np.r_[True, run[1:] != run[:-1]]
    starts = np.where(first, np.arange(e), 0)
    run_start = np.maximum.accumulate(starts)
    k_in_run = np.arange(e) - run_start

    r_e = rank_of[ds]
    j_e = r_e // P
    p_e = r_e % P
    q_e = inv_border[j_e]
    c_e = q_e % NCORES
    bp_e = q_e // NCORES

    idx_lo = np.zeros((NCORES, P, cfg.slo), np.int64)
    idx_hi = np.zeros((NCORES, P, cfg.shi), np.int64)
    msk_lo = np.full((NCORES, P, cfg.slo), NEG_MASK, np.float32)
    msk_hi = np.full((NCORES, P, cfg.shi), NEG_MASK, np.float32)

    lo = ~cs
    col = colbase_lo[bp_e[lo]] + k_in_run[lo]
    idx_lo[c_e[lo], p_e[lo], col] = ss[lo]
    msk_lo[c_e[lo], p_e[lo], col] = 0.0
    hi = cs
    col = colbase_hi[bp_e[hi]] + k_in_run[hi]
    idx_hi[c_e[hi], p_e[hi], col] = ss[hi] - hi_base
    msk_hi[c_e[hi], p_e[hi], col] = 0.0

    # dead ranks / missing blocks: one live (masked-in) lo slot at k=0 so the
    # softmax denominator stays finite (rows are discarded on unshard).
    for c in range(NCORES):
        for bp in range(cfg.nbc):
            j = bp * NCORES + c
            if j >= nb:
                msk_lo[c, :, colbase_lo[bp]] = 0.0
    dead = np.arange(n, npad)
    if dead.size:
        qd, pd = inv_border[dead // P], dead % P
        msk_lo[qd % NCORES, pd, colbase_lo[qd // NCORES]] = 0.0

    # class-select mask (1.0 -> er comes from lo stream k=0, else hi k=0)
    clsm = np.ones((NCORES, P, cfg.nbc), np.float32)
    node_blk = np.full((nb, P), -1, np.int64)
    node_blk.reshape(-1)[: n] = order
    for c in range(NCORES):
        qs = np.arange(c, nb, NCORES)               # positions owned by core c
        blocks = node_blk[border[qs]]               # [nbc_real, P]
        hi_node = (blocks >= hi_base) & (blocks >= 0)
        clsm[c, :, : blocks.shape[0]] = np.where(hi_node, 0.0, 1.0).T

    def wrap16(a):  # [P, cols] int -> [P, 8*cols] int16 gather-index layout
        cols = a.shape[1]
        stream = a.T.reshape(-1)                    # i = col*128 + p
        w = stream.reshape(-1, 16).T                # [16, cols*8]
        return np.tile(w, (8, 1)).astype(np.int16)

    # fc weights with folded attention reductions
    AL = (W.reshape(H, F, IN_FEATS) * np.asarray(attn_l)[0][:, :, None]).sum(1).T
    AR = (W.reshape(H, F, IN_FEATS) * np.asarray(attn_r)[0][:, :, None]).sum(1).T
    w_aug = np.concatenate([W.T, AL, AR], axis=1)   # [IN, 264]
    w_in = np.ascontiguousarray(
        w_aug.reshape(2, P, HF + 2 * H).astype(np.float32)
    )

    feat_pad = np.zeros((npad, IN_FEATS), np.float32)
    feat_pad[: n] = feat
    nt = npad // P
    feat_t = np.ascontiguousarray(
        feat_pad.reshape(nt, P, 2, P).transpose(0, 3, 2, 1)
    )  # [r, p(k-dim), c, n(node)]

    shared = {
        "feat_t": feat_t,
        "w_aug": w_in,
        "ident": np.eye(P, dtype=BF16),
        "bias": np.tile(np.asarray(bias, np.float32).reshape(1, HF), (P, 1)),
    }
    per_core = []
    for c in range(NCORES):
        per_core.append(
            dict(
                shared,
                idx_lo=wrap16(idx_lo[c]),
                idx_hi=wrap16(idx_hi[c]),
                msk_lo=np.ascontiguousarray(msk_lo[c]),
                msk_hi=np.ascontiguousarray(msk_hi[c]),
                clsm=np.ascontiguousarray(clsm[c]),
            )
        )
    return cfg, per_core, (order, border)


def build_program(nc, cfg, stage=99):
    import os
    dt = mybir.dt
    _skipfc = os.environ.get("KSKIPFC") == "1"
    _ngroups = int(os.environ.get("KGROUPS", "9999"))
    feat_t = nc.dram_tensor("feat_t", [cfg.nt, P, 2, P], dt.float32r, kind="ExternalInput")
    w_aug = nc.dram_tensor("w_aug", [2, P, HF + 2 * H], dt.float32r, kind="ExternalInput")
    ident = nc.dram_tensor("ident", [P, P], dt.bfloat16, kind="ExternalInput")
    bias = nc.dram_tensor("bias", [P, HF], dt.float32, kind="ExternalInput")
    idx_lo = nc.dram_tensor("idx_lo", [P, 8 * cfg.slo], dt.int16, kind="ExternalInput")
    idx_hi = nc.dram_tensor("idx_hi", [P, 8 * cfg.shi], dt.int16, kind="ExternalInput")
    msk_lo = nc.dram_tensor("msk_lo", [P, cfg.slo], dt.float32, kind="ExternalInput")
    msk_hi = nc.dram_tensor("msk_hi", [P, cfg.shi], dt.float32, kind="ExternalInput")
    clsm = nc.dram_tensor("clsm", [P, cfg.nbc], dt.float32, kind="ExternalInput")
    lo_rows = min(cfg.npad, cfg.hi_base)
    hi_rows = max(cfg.npad - cfg.hi_base, 0)
    table_lo = nc.dram_tensor("table_lo", [lo_rows, ROW], dt.bfloat16, kind="Internal")
    table_hi = (
        nc.dram_tensor("table_hi", [hi_rows, ROW], dt.bfloat16, kind="Internal")
        if hi_rows
        else table_lo
    )
    out = nc.dram_tensor("out", [cfg.nbc * P, HF], dt.float32, kind="ExternalOutput")

    NW = HF + 2 * H  # 264

    with tile.TileContext(nc) as tc:
        with ExitStack() as octx:
            cpool = octx.enter_context(tc.tile_pool(name="const", bufs=1))
            w_sb = cpool.tile([P, 2, NW], dt.float32r)
            nc.sync.dma_start(w_sb[:], w_aug[:].rearrange("c p n -> p c n"))
            id_sb = cpool.tile([P, P], dt.bfloat16)
            nc.sync.dma_start(id_sb[:], ident[:])
            bias_sb = cpool.tile([P, 1, HF], dt.float32)
            nc.sync.dma_start(bias_sb[:, 0, :], bias[:])

            # ---------------- phase A: fc -> table ----------------
            for _rep in range(int(os.environ.get("KREPEAT", "1"))):
              with ExitStack() as ctx:
                  fpool = ctx.enter_context(tc.tile_pool(name="fc_in", bufs=3))
                  rpool = ctx.enter_context(tc.tile_pool(name="fc_row", bufs=3))
                  pspool = ctx.enter_context(
                      tc.tile_pool(name="fc_ps", bufs=3, space="PSUM")
                  )
                  for r in range(0 if _skipfc else cfg.nt):
                      ft = fpool.tile([P, 2, P], dt.float32r)
                      nc.sync.dma_start(ft[:], feat_t[r])
                      ps = pspool.tile([P, NW], dt.float32)
                      nc.tensor.matmul(
                          ps[:],
                          lhsT=ft[:, 0, :],
                          rhs=w_sb[:, 0, :],
                          start=True, stop=False,
                      )
                      nc.tensor.matmul(
                          ps[:],
                          lhsT=ft[:, 1, :],
                          rhs=w_sb[:, 1, :],
                          start=False, stop=True,
                      )
                      row = rpool.tile([P, ROWU], dt.bfloat16)
                      # [0:256]=h  [256:264]=el_hi,er_hi
                      nc.scalar.activation(
                          row[:, 0 : ELO_OFF], ps[:, 0:NW], mybir.ActivationFunctionType.Copy
                      )
                      # [264:272] = (el,er) - bf16(el,er)
                      nc.vector.tensor_tensor(
                          out=row[:, ELO_OFF : ELO_OFF + 8],
                          in0=ps[:, HF:NW],
                          in1=row[:, EHI_OFF : EHI_OFF + 8],
                          op=mybir.AluOpType.subtract,
                      )
                      if r * P < cfg.hi_base:
                          nc.sync.dma_start(
                              table_lo[r * P : (r + 1) * P, 0:ROWU], row[:]
                          )
                      else:
                          rr = r * P - cfg.hi_base
                          nc.sync.dma_start(table_hi[rr : rr + P, 0:ROWU], row[:])
                      if stage == 1 and r < cfg.nbc:
                          orow = rpool.tile([P, HF], dt.float32, tag="dbg")
                          nc.vector.tensor_copy(orow[:], row[:, 0:HF])
                          nc.sync.dma_start(out[r * P : (r + 1) * P, :], orow[:])

              if stage >= 2:
                  _phase_b(nc, tc, cfg, stage, w_sb, id_sb, bias_sb,
                           idx_lo, idx_hi, msk_lo, msk_hi, clsm,
                           (table_lo, table_hi), out, ngroups=_ngroups)
    nc.compile()


def _phase_b(nc, tc, cfg, stage, w_sb, id_sb, bias_sb,
             idx_lo, idx_hi, msk_lo, msk_hi, clsm, tables, out, ngroups=9999):
    import os
    dt = mybir.dt
    table_lo, table_hi = tables
    if True:
        if True:
            if os.environ.get("KBAR"):
                tc.strict_bb_all_engine_barrier()

            # ---------------- phase B: gather + attention + aggregate ----------------
            with ExitStack() as ctx:
                gpool = ctx.enter_context(tc.tile_pool(name="gath", bufs=2))
                ipool = ctx.enter_context(tc.tile_pool(name="gidx", bufs=2))
                mpool = ctx.enter_context(tc.tile_pool(name="gmask", bufs=2))
                spool = ctx.enter_context(tc.tile_pool(name="scores", bufs=2))
                opool = ctx.enter_context(tc.tile_pool(name="outs", bufs=2))
                mbpool = ctx.enter_context(tc.tile_pool(name="mrows", bufs=2))
                pspool = ctx.enter_context(
                    tc.tile_pool(name="agg_ps", bufs=2, space="PSUM")
                )

                GCOLS = 56  # max ELL columns per dma_gather call (idx ring limit)
                for bp0, g, dlo, dhi, b_lo, b_hi in cfg.groups[:ngroups]:
                    cl = g * dlo          # lo columns this group
                    ch = g * dhi
                    il = ipool.tile([P, 8 * cl], dt.int16, tag="il")
                    nc.sync.dma_start(il[:], idx_lo[:, 8 * b_lo : 8 * (b_lo + cl)])
                    gl = gpool.tile([P, cl, ROWU], dt.bfloat16, tag="gl")
                    for c0 in range(0, cl, GCOLS):
                        cc = min(GCOLS, cl - c0)
                        _emit_gather(
                            nc, gl[:, c0 : c0 + cc, :], table_lo[:, 0:ROWU],
                            il[:, 8 * c0 : 8 * (c0 + cc)], P * cc,
                        )
                    ih = ipool.tile([P, 8 * ch], dt.int16, tag="ih")
                    nc.sync.dma_start(ih[:], idx_hi[:, 8 * b_hi : 8 * (b_hi + ch)])
                    gh = gpool.tile([P, ch, ROWU], dt.bfloat16, tag="gh")
                    for c0 in range(0, ch, GCOLS):
                        cc = min(GCOLS, ch - c0)
                        _emit_gather(
                            nc, gh[:, c0 : c0 + cc, :], table_hi[:, 0:ROWU],
                            ih[:, 8 * c0 : 8 * (c0 + cc)], P * cc,
                        )
                    ml = mpool.tile([P, cl], dt.float32, tag="ml")
                    if not os.environ.get("KNOMASK"):
                        nc.sync.dma_start(ml[:], msk_lo[:, b_lo : b_lo + cl])
                    mh = mpool.tile([P, ch], dt.float32, tag="mh")
                    if not os.environ.get("KNOMASK"):
                        nc.sync.dma_start(mh[:], msk_hi[:, b_hi : b_hi + ch])
                    cm = mpool.tile([P, g], dt.float32, tag="cm")
                    if not os.environ.get("KNOCM"):
                        nc.sync.dma_start(cm[:], clsm[:, bp0 : bp0 + g])

                    if stage == 2:  # gathers only
                        if os.environ.get("KNOOT"):
                            continue
                        ot = opool.tile([P, g, HF], dt.float32, tag="ot")
                        nc.vector.tensor_copy(ot[:], gl[:, 0:g, 0:HF])
                        nc.vector.tensor_tensor(
                            out=ot[:], in0=ot[:], in1=gh[:, 0:g, 0:HF],
                            op=mybir.AluOpType.add,
                        )
                        nc.sync.dma_start(
                            out[bp0 * P : (bp0 + g) * P, :].rearrange(
                                "(g p) n -> p g n", p=P
                            ),
                            ot[:],
                        )
                        continue

                    glv = gl[:].rearrange("p (g d) r -> p g d r", g=g)
                    ghv = gh[:].rearrange("p (g d) r -> p g d r", g=g)

                    # er select from each block's k=0 self-edge row
                    er = spool.tile([P, g, 1, H], dt.float32, tag="er")
                    erh = spool.tile([P, g, H], dt.float32, tag="erh")
                    nc.vector.tensor_tensor(
                        out=er[:, :, 0, :],
                        in0=glv[:, :, 0, EHI_OFF + H : EHI_OFF + 2 * H],
                        in1=glv[:, :, 0, ELO_OFF + H : ELO_OFF + 2 * H],
                        op=mybir.AluOpType.add,
                    )
                    nc.vector.tensor_tensor(
                        out=erh[:],
                        in0=ghv[:, :, 0, EHI_OFF + H : EHI_OFF + 2 * H],
                        in1=ghv[:, :, 0, ELO_OFF + H : ELO_OFF + 2 * H],
                        op=mybir.AluOpType.add,
                    )
                    # er := erh + (er - erh) * clsm   (arithmetic select)
                    nc.vector.tensor_tensor(
                        out=er[:, :, 0, :], in0=er[:, :, 0, :], in1=erh[:],
                        op=mybir.AluOpType.subtract,
                    )
                    nc.vector.tensor_tensor(
                        out=er[:, :, 0, :], in0=er[:, :, 0, :],
                        in1=cm[:].to_broadcast([P, g, H]),
                        op=mybir.AluOpType.mult,
                    )
                    nc.vector.tensor_tensor(
                        out=er[:, :, 0, :], in0=er[:, :, 0, :], in1=erh[:],
                        op=mybir.AluOpType.add,
                    )

                    dn = spool.tile([P, g, H], dt.float32, tag="dn")
                    mbs = []

                    for side, gv, cols, d, mt in (
                        (0, glv, cl, dlo, ml),
                        (1, ghv, ch, dhi, mh),
                    ):
                        ex = spool.tile([P, g, d, H], dt.float32, tag=f"ex{side}")
                        # e = el_hi + el_lo
                        nc.vector.tensor_tensor(
                            out=ex[:],
                            in0=gv[:, :, :, EHI_OFF : EHI_OFF + H],
                            in1=gv[:, :, :, ELO_OFF : ELO_OFF + H],
                            op=mybir.AluOpType.add,
                        )
                        # += er
                        nc.vector.tensor_tensor(
                            out=ex[:],
                            in0=ex[:],
                            in1=er[:].to_broadcast([P, g, d, H]),
                            op=mybir.AluOpType.add,
                        )
                        # leaky relu: e = max(e, 0.2*e)
                        lr = spool.tile([P, g, d, H], dt.float32, tag=f"lr{side}")
                        nc.vector.tensor_scalar_mul(lr[:], ex[:], NEG)
                        nc.vector.tensor_tensor(
                            out=ex[:], in0=ex[:], in1=lr[:],
                            op=mybir.AluOpType.max,
                        )
                        # += pad mask
                        nc.vector.tensor_tensor(
                            out=ex[:],
                            in0=ex[:],
                            in1=mt[:]
                            .rearrange("p (g d) -> p g d", g=g)
                            .to_broadcast([P, g, d, H]),
                            op=mybir.AluOpType.add,
                        )
                        # exp
                        nc.scalar.activation(
                            ex[:], ex[:], mybir.ActivationFunctionType.Exp
                        )
                        # denominators (partial): reduce over d
                        dnp = dn if side == 0 else spool.tile(
                            [P, g, H], dt.float32, tag="dnh"
                        )
                        nc.vector.tensor_reduce(
                            out=dnp[:],
                            in_=ex[:].rearrange("p g d h -> p g h d"),
                            axis=mybir.AxisListType.X,
                            op=mybir.AluOpType.add,
                        )
                        if side == 1:
                            nc.vector.tensor_tensor(
                                out=dn[:], in0=dn[:], in1=dnp[:], op=mybir.AluOpType.add
                            )
                        if stage == 3:
                            continue
                        exb = spool.tile([P, g, d, H], dt.bfloat16, tag=f"exb{side}")
                        nc.vector.tensor_copy(exb[:], ex[:])
                        mb = mbpool.tile([P, g * d, HF], dt.bfloat16, tag=f"mb{side}")
                        if os.environ.get("KNOMMUL"):
                            mbs.append(mb)
                            continue
                        nc.vector.tensor_tensor(
                            out=mb[:].rearrange("p s (h f) -> p s h f", f=F),
                            in0=gv[:, :, :, 0:HF].rearrange(
                                "p g d (h f) -> p (g d) h f", f=F
                            ),
                            in1=exb[:]
                            .rearrange("p g d h -> p (g d) h")
                            .to_broadcast([P, g * d, H, F]),
                            op=mybir.AluOpType.mult,
                        )
                        mbs.append(mb)

                    rcp = spool.tile([P, g, H], dt.float32, tag="rcp")
                    nc.vector.reciprocal(rcp[:], dn[:])
                    ot = opool.tile([P, g, HF], dt.float32, tag="ot")
                    # aggregate per block: identity-matmul over weighted rows
                    for gi in range(g if not os.environ.get("KNOMM") else 0):
                        ps_b = pspool.tile([P, HF], dt.float32, tag="ps")
                        nmm = dlo + dhi
                        i = 0
                        for mb, d in zip(mbs, (dlo, dhi)):
                            for k in range(d):
                                nc.tensor.matmul(
                                    ps_b[:],
                                    lhsT=id_sb[:],
                                    rhs=mb[:, gi * d + k, :],
                                    start=(i == 0),
                                    stop=(i == nmm - 1),
                                )
                                i += 1
                        nc.vector.tensor_tensor(
                            out=ot[:, gi, :].rearrange("p (h f) -> p h f", f=F),
                            in0=ps_b[:].rearrange("p (h f) -> p h f", f=F),
                            in1=rcp[:, gi, :].to_broadcast([P, H, F]),
                            op=mybir.AluOpType.mult,
                        )
                    nc.vector.tensor_tensor(
                        out=ot[:],
                        in0=ot[:],
                        in1=bias_sb[:].to_broadcast([P, g, HF]),
                        op=mybir.AluOpType.add,
                    )
                    nc.sync.dma_start(
                        out[bp0 * P : (bp0 + g) * P, :].rearrange(
                            "(g p) n -> p g n", p=P
                        ),
                        ot[:],
                    )


def kernel(feat, W, attn_l, attn_r, bias, src, dst):
    n = feat.shape[0]
    feat = np.asarray(feat, np.float32)
    W = np.asarray(W, np.float32)
    bias_np = np.asarray(bias, np.float32)
    cfg, per_core, (order, border) = host_prep(
        feat, W, np.asarray(attn_l, np.float32), np.asarray(attn_r, np.float32),
        bias_np, np.asarray(src), np.asarray(dst), n, hi_base=HI_BASE,
    )
    import os
    stage = int(os.environ.get("KSTAGE", "99"))
    nc = bacc.Bacc("TRN2", target_bir_lowering=False, debug=False, num_devices=NCORES)
    build_program(nc, cfg, stage=stage)
    res = run_bass_kernel_spmd(
        nc, per_core, core_ids=list(range(NCORES)), trace=TRACE
    )
    globals()["LAST_RESULTS"] = res
    outs = [res.results[c]["out"] for c in range(NCORES)]  # [nbc*128, 256] each

    full = np.zeros((cfg.npad, HF), np.float32)
    nb = cfg.npad // P
    for c in range(NCORES):
        o = outs[c].reshape(cfg.nbc, P, HF)
        js = border[np.arange(c, nb, NCORES)]  # global blocks owned by core c
        full[(js[:, None] * P + np.arange(P)).reshape(-1)] = o[: len(js)].reshape(
            -1, HF
        )
    # rank r holds node order[r]
    result = np.zeros((n, H, F), np.float32)
    result[order] = full[: n].reshape(n, H, F)
    return result



# revision 3
# speedup vs baseline: 4.0003x; 4.0003x over previous
"""Distributed GAT layer on 8 Trainium2 NeuronCores (Bass/Tile).

Strategy (dst-sharded, degree-aligned ELL):
  * fc projection replicated on every core: h = feat @ W.T plus folded
    attention reductions el = feat @ AL, er = feat @ AR computed in the same
    matmul; rows written to a DRAM table [h bf16 | el hi/lo | er hi/lo | pad]
    of 768 B per node.
  * dst nodes globally sorted by (lo-degree, hi-degree), packed into blocks
    of 128, blocks dealt round-robin to the 8 cores so every core runs the
    identical instruction stream on identically-shaped slot arrays.
  * per block, incoming edges live in an ELL slot array [128 dst x D cols];
    src rows are fetched with dma_gather (two streams, since gather indices
    are int16: table rows < 32768 and the rest), scores computed on DVE/ACT
    (exp without max-subtraction -- magnitudes are bounded), softmax denom
    via strided reduce, weighted aggregation via identity-matmul PSUM
    accumulation, then scale + bias.
"""

import math
from contextlib import ExitStack
from dataclasses import dataclass, field

import ml_dtypes
import numpy as np

import concourse.bass as bass
import concourse.tile as tile
from concourse import bacc, mybir
from concourse.bass_utils import run_bass_kernel_spmd

P = 128
NCORES = 8
H, F = 4, 64
HF = H * F            # 256
IN_FEATS = 256
ROW = 384             # bf16 elements per table row stride (768 B)
ROWU = 272            # payload elements actually gathered (544 B)
H_OFF = 0             # [0:256]   h bf16
EHI_OFF = 256         # [256:260] el hi, [260:264] er hi
ELO_OFF = 264         # [264:268] el lo, [268:272] er lo
NEG = 0.2
HI_BASE = 32768
NEG_MASK = -1.0e30
BF16 = ml_dtypes.bfloat16
TRACE = False
LAST_RESULTS = None


@dataclass
class Cfg:
    n: int
    npad: int
    nt: int                      # fc tiles = npad // 128
    nbc: int                     # block-positions per core
    groups: list = field(default_factory=list)   # (bp0, g, dlo, dhi, base_lo, base_hi)
    slo: int = 0                 # total lo slot-columns per core
    shi: int = 0
    hi_base: int = HI_BASE


def _emit_gather(nc, out_ap, in_ap, idxs_ap, num_idxs, queue_num=0,
                 elem_size=None, elem_step=None):
    """dma_gather with payload < row stride (bass asserts %256 on payload,
    but only the stride is 256B-quantized in the descriptor format --
    verified on HW)."""
    import concourse.ap_utils as ap_utils
    from concourse.bass import MemorySpace
    eng = nc.gpsimd
    elem_size = ROWU if elem_size is None else elem_size
    elem_step = ROW if elem_step is None else elem_step
    assert idxs_ap.dtype == mybir.dt.int16
    assert in_ap.dtype == out_ap.dtype
    assert in_ap.space == MemorySpace.DRAM
    assert idxs_ap.space == MemorySpace.SBUF and out_ap.space == MemorySpace.SBUF
    assert ap_utils.ap_is_contiguous(out_ap.ap[1:])
    assert ap_utils.ap_is_contiguous(idxs_ap.ap[1:])
    assert in_ap.ap[-1][1] == elem_size == out_ap.ap[-1][1]
    assert out_ap.ap[0][1] * out_ap.ap[1][1] == num_idxs
    assert in_ap.ap[0][0] == elem_step
    stride_bytes = elem_step * mybir.dt.size(in_ap.dtype)
    _in_ap = eng.lower_ap_dma(in_ap, for_custom_bir_dma=True)
    _idxs_ap = eng.lower_ap(idxs_ap)
    _out_ap = eng.lower_ap(out_ap)
    return eng.add_instruction(
        mybir.InstDMAGatherAnt(
            name=eng.bass.get_next_instruction_name(),
            ins=[*_in_ap, _idxs_ap, eng.lower_val_access(eng.to_reg(num_idxs))],
            outs=[_out_ap],
            transpose=False, num_idxs=num_idxs, elem_size=elem_size,
            stride_bytes_256=stride_bytes // 256, gen_mode=0,
            single_packet=False, queue_num=queue_num,
            sbuf_tokens_per_rank=0, sbuf_free_dim_per_rank=0,
            sbuf_free_dim_pad_per_rank=0, sbuf_byte_offset=0,
        )
    )


def build_cfg(deg_lo_blk, deg_hi_blk, n, npad, group_max=4):
    """deg_*_blk: [NB] per-block max degree arrays (global block order)."""
    nb = npad // P
    nbc = (nb + NCORES - 1) // NCORES
    # per-position max over the cores' blocks at that position
    dlo_p = np.ones(nbc, np.int64)
    dhi_p = np.ones(nbc, np.int64)
    np.maximum.at(dlo_p, np.arange(nb) // NCORES, deg_lo_blk)
    np.maximum.at(dhi_p, np.arange(nb) // NCORES, deg_hi_blk)

    cfg = Cfg(n=n, npad=npad, nt=npad // P, nbc=nbc)
    base_lo = base_hi = 0
    bp = 0
    budget = 38 * 1024  # bytes/partition for one gather buffer set
    while bp < nbc:
        g = 1
        while (
            bp + g < nbc
            and g < group_max
            and (g + 1)
            * (max(dlo_p[bp : bp + g + 1]) + max(dhi_p[bp : bp + g + 1]))
            * (2 * ROWU)
            <= budget
        ):
            g += 1
        dlo = int(max(dlo_p[bp : bp + g]))
        dhi = int(max(dhi_p[bp : bp + g]))
        cfg.groups.append((bp, g, dlo, dhi, base_lo, base_hi))
        base_lo += g * dlo
        base_hi += g * dhi
        bp += g
    cfg.slo = base_lo
    cfg.shi = base_hi
    return cfg


def host_prep(feat, W, attn_l, attn_r, bias, src, dst, n, hi_base=HI_BASE):
    """Returns (cfg, shared_map, per_core_maps, order) ."""
    e = src.shape[0]
    npad = ((n + P * NCORES - 1) // (P * NCORES)) * P * NCORES  # multiple of 1024
    nb = npad // P

    src = np.asarray(src, np.int64)
    dst = np.asarray(dst, np.int64)
    cls_e = src >= hi_base                       # per-edge: hi stream?
    deg_lo = np.bincount(dst[~cls_e], minlength=n)
    deg_hi = np.bincount(dst[cls_e], minlength=n)

    order = np.lexsort((deg_hi, deg_lo))          # rank -> node
    rank_of = np.empty(n, np.int64)
    rank_of[order] = np.arange(n)

    # block stats in global block order; re-sort blocks by their degree
    # profile so blocks dealt to the same position pad alike
    dlo_r = np.zeros(npad, np.int64)
    dhi_r = np.zeros(npad, np.int64)
    dlo_r[: n] = deg_lo[order]
    dhi_r[: n] = deg_hi[order]
    blk_max_lo = dlo_r.reshape(nb, P).max(1)
    blk_max_hi = dhi_r.reshape(nb, P).max(1)
    border = np.lexsort((blk_max_hi, blk_max_lo, blk_max_hi // 2))  # q -> j
    inv_border = np.empty(nb, np.int64)
    inv_border[border] = np.arange(nb)
    cfg = build_cfg(blk_max_lo[border], blk_max_hi[border], n, npad)
    cfg.hi_base = hi_base

    # per-bp column base within the core-wide slot arrays
    colbase_lo = np.zeros(cfg.nbc, np.int64)
    colbase_hi = np.zeros(cfg.nbc, np.int64)
    dlo_of_bp = np.zeros(cfg.nbc, np.int64)
    dhi_of_bp = np.zeros(cfg.nbc, np.int64)
    for bp0, g, dlo, dhi, b_lo, b_hi in cfg.groups:
        for gi in range(g):
            colbase_lo[bp0 + gi] = b_lo + gi * dlo
            colbase_hi[bp0 + gi] = b_hi + gi * dhi
            dlo_of_bp[bp0 + gi] = dlo
            dhi_of_bp[bp0 + gi] = dhi

    # order edges by (dst, class, non-self) so the self edge heads its class run
    self_e = src == dst
    es = np.lexsort((~self_e, cls_e, dst))
    ds, ss, cs = dst[es], src[es], cls_e[es]
    run = ds * 2 + cs
    first = np.r_[True, run[1:] != run[:-1]]
    starts = np.where(first, np.arange(e), 0)
    run_start = np.maximum.accumulate(starts)
    k_in_run = np.arange(e) - run_start

    r_e = rank_of[ds]
    j_e = r_e // P
    p_e = r_e % P
    q_e = inv_border[j_e]
    c_e = q_e % NCORES
    bp_e = q_e // NCORES

    idx_lo = np.zeros((NCORES, P, cfg.slo), np.int64)
    idx_hi = np.zeros((NCORES, P, cfg.shi), np.int64)
    msk_lo = np.full((NCORES, P, cfg.slo), NEG_MASK, np.float32)
    msk_hi = np.full((NCORES, P, cfg.shi), NEG_MASK, np.float32)

    lo = ~cs
    col = colbase_lo[bp_e[lo]] + k_in_run[lo]
    idx_lo[c_e[lo], p_e[lo], col] = ss[lo]
    msk_lo[c_e[lo], p_e[lo], col] = 0.0
    hi = cs
    col = colbase_hi[bp_e[hi]] + k_in_run[hi]
    idx_hi[c_e[hi], p_e[hi], col] = ss[hi] - hi_base
    msk_hi[c_e[hi], p_e[hi], col] = 0.0

    # dead ranks / missing blocks: one live (masked-in) lo slot at k=0 so the
    # softmax denominator stays finite (rows are discarded on unshard).
    for c in range(NCORES):
        for bp in range(cfg.nbc):
            j = bp * NCORES + c
            if j >= nb:
                msk_lo[c, :, colbase_lo[bp]] = 0.0
    dead = np.arange(n, npad)
    if dead.size:
        qd, pd = inv_border[dead // P], dead % P
        msk_lo[qd % NCORES, pd, colbase_lo[qd // NCORES]] = 0.0

    # class-select mask (1.0 -> er comes from lo stream k=0, else hi k=0)
    clsm = np.ones((NCORES, P, cfg.nbc), np.float32)
    node_blk = np.full((nb, P), -1, np.int64)
    node_blk.reshape(-1)[: n] = order
    for c in range(NCORES):
        qs = np.arange(c, nb, NCORES)               # positions owned by core c
        blocks = node_blk[border[qs]]               # [nbc_real, P]
        hi_node = (blocks >= hi_base) & (blocks >= 0)
        clsm[c, :, : blocks.shape[0]] = np.where(hi_node, 0.0, 1.0).T

    def wrap16(a):  # [P, cols] int -> [P, 8*cols] int16 gather-index layout
        cols = a.shape[1]
        stream = a.T.reshape(-1)                    # i = col*128 + p
        w = stream.reshape(-1, 16).T                # [16, cols*8]
        return np.tile(w, (8, 1)).astype(np.int16)

    # fc weights with folded attention reductions
    AL = (W.reshape(H, F, IN_FEATS) * np.asarray(attn_l)[0][:, :, None]).sum(1).T
    AR = (W.reshape(H, F, IN_FEATS) * np.asarray(attn_r)[0][:, :, None]).sum(1).T
    w_aug = np.concatenate([W.T, AL, AR], axis=1)   # [IN, 264]
    w_in = np.ascontiguousarray(
        w_aug.reshape(2, P, HF + 2 * H).astype(np.float32)
    )

    feat_pad = np.zeros((npad, IN_FEATS), np.float32)
    feat_pad[: n] = feat
    nt = npad // P
    feat_t = np.ascontiguousarray(
        feat_pad.reshape(nt, P, 2, P).transpose(0, 3, 2, 1)
    )  # [r, p(k-dim), c, n(node)]

    shared = {
        "feat_t": feat_t,
        "w_aug": w_in,
        "ident": np.eye(P, dtype=BF16),
        "bias": np.tile(np.asarray(bias, np.float32).reshape(1, HF), (P, 1)),
    }
    per_core = []
    for c in range(NCORES):
        per_core.append(
            dict(
                shared,
                idx_lo=wrap16(idx_lo[c]),
                idx_hi=wrap16(idx_hi[c]),
                msk_lo=np.ascontiguousarray(msk_lo[c]),
                msk_hi=np.ascontiguousarray(msk_hi[c]),
                clsm=np.ascontiguousarray(clsm[c]),
            )
        )
    return cfg, per_core, (order, border)


def build_program(nc, cfg, stage=99):
    import os
    dt = mybir.dt
    _skipfc = os.environ.get("KSKIPFC") == "1"
    _ngroups = int(os.environ.get("KGROUPS", "9999"))
    feat_t = nc.dram_tensor("feat_t", [cfg.nt, P, 2, P], dt.float32r, kind="ExternalInput")
    w_aug = nc.dram_tensor("w_aug", [2, P, HF + 2 * H], dt.float32r, kind="ExternalInput")
    ident = nc.dram_tensor("ident", [P, P], dt.bfloat16, kind="ExternalInput")
    bias = nc.dram_tensor("bias", [P, HF], dt.float32, kind="ExternalInput")
    idx_lo = nc.dram_tensor("idx_lo", [P, 8 * cfg.slo], dt.int16, kind="ExternalInput")
    idx_hi = nc.dram_tensor("idx_hi", [P, 8 * cfg.shi], dt.int16, kind="ExternalInput")
    msk_lo = nc.dram_tensor("msk_lo", [P, cfg.slo], dt.float32, kind="ExternalInput")
    msk_hi = nc.dram_tensor("msk_hi", [P, cfg.shi], dt.float32, kind="ExternalInput")
    clsm = nc.dram_tensor("clsm", [P, cfg.nbc], dt.float32, kind="ExternalInput")
    lo_rows = min(cfg.npad, cfg.hi_base)
    hi_rows = max(cfg.npad - cfg.hi_base, 0)
    table_lo = nc.dram_tensor("table_lo", [lo_rows, ROW], dt.bfloat16, kind="Internal")
    table_hi = (
        nc.dram_tensor("table_hi", [hi_rows, ROW], dt.bfloat16, kind="Internal")
        if hi_rows
        else table_lo
    )
    out = nc.dram_tensor("out", [cfg.nbc * P, HF], dt.float32, kind="ExternalOutput")

    NW = HF + 2 * H  # 264

    with tile.TileContext(nc) as tc:
        with ExitStack() as octx:
            cpool = octx.enter_context(tc.tile_pool(name="const", bufs=1))
            w_sb = cpool.tile([P, 2, NW], dt.float32r)
            nc.sync.dma_start(w_sb[:], w_aug[:].rearrange("c p n -> p c n"))
            id_sb = cpool.tile([P, P], dt.bfloat16)
            nc.sync.dma_start(id_sb[:], ident[:])
            bias_sb = cpool.tile([P, 1, HF], dt.float32)
            nc.sync.dma_start(bias_sb[:, 0, :], bias[:])

            # ---------------- phase A: fc -> table ----------------
            for _rep in range(int(os.environ.get("KREPEAT", "1"))):
              with ExitStack() as ctx:
                  fpool = ctx.enter_context(tc.tile_pool(name="fc_in", bufs=3))
                  rpool = ctx.enter_context(tc.tile_pool(name="fc_row", bufs=3))
                  pspool = ctx.enter_context(
                      tc.tile_pool(name="fc_ps", bufs=3, space="PSUM")
                  )
                  for r in range(0 if _skipfc else cfg.nt):
                      ft = fpool.tile([P, 2, P], dt.float32r)
                      nc.sync.dma_start(ft[:], feat_t[r])
                      ps = pspool.tile([P, NW], dt.float32)
                      nc.tensor.matmul(
                          ps[:],
                          lhsT=ft[:, 0, :],
                          rhs=w_sb[:, 0, :],
                          start=True, stop=False,
                      )
                      nc.tensor.matmul(
                          ps[:],
                          lhsT=ft[:, 1, :],
                          rhs=w_sb[:, 1, :],
                          start=False, stop=True,
                      )
                      row = rpool.tile([P, ROWU], dt.bfloat16)
                      # [0:256]=h  [256:264]=el_hi,er_hi
                      nc.scalar.activation(
                          row[:, 0 : ELO_OFF], ps[:, 0:NW], mybir.ActivationFunctionType.Copy
                      )
                      # [264:272] = (el,er) - bf16(el,er)
                      nc.vector.tensor_tensor(
                          out=row[:, ELO_OFF : ELO_OFF + 8],
                          in0=ps[:, HF:NW],
                          in1=row[:, EHI_OFF : EHI_OFF + 8],
                          op=mybir.AluOpType.subtract,
                      )
                      if r * P < cfg.hi_base:
                          nc.sync.dma_start(
                              table_lo[r * P : (r + 1) * P, 0:ROWU], row[:]
                          )
                      else:
                          rr = r * P - cfg.hi_base
                          nc.sync.dma_start(table_hi[rr : rr + P, 0:ROWU], row[:])
                      if stage == 1 and r < cfg.nbc:
                          orow = rpool.tile([P, HF], dt.float32, tag="dbg")
                          nc.vector.tensor_copy(orow[:], row[:, 0:HF])
                          nc.sync.dma_start(out[r * P : (r + 1) * P, :], orow[:])

              if stage >= 2:
                  _phase_b(nc, tc, cfg, stage, w_sb, id_sb, bias_sb,
                           idx_lo, idx_hi, msk_lo, msk_hi, clsm,
                           (table_lo, table_hi), out, ngroups=_ngroups)
    nc.compile()


def _phase_b(nc, tc, cfg, stage, w_sb, id_sb, bias_sb,
             idx_lo, idx_hi, msk_lo, msk_hi, clsm, tables, out, ngroups=9999):
    import os
    dt = mybir.dt
    table_lo, table_hi = tables
    if True:
        if True:
            if os.environ.get("KBAR"):
                tc.strict_bb_all_engine_barrier()

            # ---------------- phase B: gather + attention + aggregate ----------------
            with ExitStack() as ctx:
                gpool = ctx.enter_context(tc.tile_pool(name="gath", bufs=2))
                ipool = ctx.enter_context(tc.tile_pool(name="gidx", bufs=2))
                mpool = ctx.enter_context(tc.tile_pool(name="gmask", bufs=2))
                spool = ctx.enter_context(tc.tile_pool(name="scores", bufs=2))
                opool = ctx.enter_context(tc.tile_pool(name="outs", bufs=2))
                mbpool = ctx.enter_context(tc.tile_pool(name="mrows", bufs=2))
                pspool = ctx.enter_context(
                    tc.tile_pool(name="agg_ps", bufs=2, space="PSUM")
                )

                GCOLS = int(os.environ.get("KGCOLS", "28"))  # ELL columns per dma_gather call
                NQ = int(os.environ.get("KNQ", "4"))  # SWDGE queues to round-robin
                qctr = [0]
                for bp0, g, dlo, dhi, b_lo, b_hi in cfg.groups[:ngroups]:
                    cl = g * dlo          # lo columns this group
                    ch = g * dhi
                    il = ipool.tile([P, 8 * cl], dt.int16, tag="il")
                    nc.sync.dma_start(il[:], idx_lo[:, 8 * b_lo : 8 * (b_lo + cl)])
                    gl = gpool.tile([P, cl, ROWU], dt.bfloat16, tag="gl")
                    for c0 in range(0, cl, GCOLS):
                        cc = min(GCOLS, cl - c0)
                        _emit_gather(
                            nc, gl[:, c0 : c0 + cc, :], table_lo[:, 0:ROWU],
                            il[:, 8 * c0 : 8 * (c0 + cc)], P * cc,
                            queue_num=qctr[0] % NQ,
                        )
                        qctr[0] += 1
                    ih = ipool.tile([P, 8 * ch], dt.int16, tag="ih")
                    nc.sync.dma_start(ih[:], idx_hi[:, 8 * b_hi : 8 * (b_hi + ch)])
                    gh = gpool.tile([P, ch, ROWU], dt.bfloat16, tag="gh")
                    for c0 in range(0, ch, GCOLS):
                        cc = min(GCOLS, ch - c0)
                        _emit_gather(
                            nc, gh[:, c0 : c0 + cc, :], table_hi[:, 0:ROWU],
                            ih[:, 8 * c0 : 8 * (c0 + cc)], P * cc,
                            queue_num=qctr[0] % NQ,
                        )
                        qctr[0] += 1
                    ml = mpool.tile([P, cl], dt.float32, tag="ml")
                    if not os.environ.get("KNOMASK"):
                        nc.sync.dma_start(ml[:], msk_lo[:, b_lo : b_lo + cl])
                    mh = mpool.tile([P, ch], dt.float32, tag="mh")
                    if not os.environ.get("KNOMASK"):
                        nc.sync.dma_start(mh[:], msk_hi[:, b_hi : b_hi + ch])
                    cm = mpool.tile([P, g], dt.float32, tag="cm")
                    if not os.environ.get("KNOCM"):
                        nc.sync.dma_start(cm[:], clsm[:, bp0 : bp0 + g])

                    if stage == 2:  # gathers only
                        if os.environ.get("KNOOT"):
                            continue
                        ot = opool.tile([P, g, HF], dt.float32, tag="ot")
                        nc.vector.tensor_copy(ot[:], gl[:, 0:g, 0:HF])
                        nc.vector.tensor_tensor(
                            out=ot[:], in0=ot[:], in1=gh[:, 0:g, 0:HF],
                            op=mybir.AluOpType.add,
                        )
                        nc.sync.dma_start(
                            out[bp0 * P : (bp0 + g) * P, :].rearrange(
                                "(g p) n -> p g n", p=P
                            ),
                            ot[:],
                        )
                        continue

                    glv = gl[:].rearrange("p (g d) r -> p g d r", g=g)
                    ghv = gh[:].rearrange("p (g d) r -> p g d r", g=g)

                    # er select from each block's k=0 self-edge row
                    er = spool.tile([P, g, 1, H], dt.float32, tag="er")
                    erh = spool.tile([P, g, H], dt.float32, tag="erh")
                    nc.vector.tensor_tensor(
                        out=er[:, :, 0, :],
                        in0=glv[:, :, 0, EHI_OFF + H : EHI_OFF + 2 * H],
                        in1=glv[:, :, 0, ELO_OFF + H : ELO_OFF + 2 * H],
                        op=mybir.AluOpType.add,
                    )
                    nc.vector.tensor_tensor(
                        out=erh[:],
                        in0=ghv[:, :, 0, EHI_OFF + H : EHI_OFF + 2 * H],
                        in1=ghv[:, :, 0, ELO_OFF + H : ELO_OFF + 2 * H],
                        op=mybir.AluOpType.add,
                    )
                    # er := erh + (er - erh) * clsm   (arithmetic select)
                    nc.vector.tensor_tensor(
                        out=er[:, :, 0, :], in0=er[:, :, 0, :], in1=erh[:],
                        op=mybir.AluOpType.subtract,
                    )
                    nc.vector.tensor_tensor(
                        out=er[:, :, 0, :], in0=er[:, :, 0, :],
                        in1=cm[:].to_broadcast([P, g, H]),
                        op=mybir.AluOpType.mult,
                    )
                    nc.vector.tensor_tensor(
                        out=er[:, :, 0, :], in0=er[:, :, 0, :], in1=erh[:],
                        op=mybir.AluOpType.add,
                    )

                    dn = spool.tile([P, g, H], dt.float32, tag="dn")
                    mbs = []

                    for side, gv, cols, d, mt in (
                        (0, glv, cl, dlo, ml),
                        (1, ghv, ch, dhi, mh),
                    ):
                        ex = spool.tile([P, g, d, H], dt.float32, tag=f"ex{side}")
                        # e = el_hi + el_lo
                        nc.vector.tensor_tensor(
                            out=ex[:],
                            in0=gv[:, :, :, EHI_OFF : EHI_OFF + H],
                            in1=gv[:, :, :, ELO_OFF : ELO_OFF + H],
                            op=mybir.AluOpType.add,
                        )
                        # += er
                        nc.vector.tensor_tensor(
                            out=ex[:],
                            in0=ex[:],
                            in1=er[:].to_broadcast([P, g, d, H]),
                            op=mybir.AluOpType.add,
                        )
                        # leaky relu: e = max(e, 0.2*e)
                        lr = spool.tile([P, g, d, H], dt.float32, tag=f"lr{side}")
                        nc.vector.tensor_scalar_mul(lr[:], ex[:], NEG)
                        nc.vector.tensor_tensor(
                            out=ex[:], in0=ex[:], in1=lr[:],
                            op=mybir.AluOpType.max,
                        )
                        # += pad mask
                        nc.vector.tensor_tensor(
                            out=ex[:],
                            in0=ex[:],
                            in1=mt[:]
                            .rearrange("p (g d) -> p g d", g=g)
                            .to_broadcast([P, g, d, H]),
                            op=mybir.AluOpType.add,
                        )
                        # exp
                        nc.scalar.activation(
                            ex[:], ex[:], mybir.ActivationFunctionType.Exp
                        )
                        # denominators (partial): reduce over d
                        dnp = dn if side == 0 else spool.tile(
                            [P, g, H], dt.float32, tag="dnh"
                        )
                        nc.vector.tensor_reduce(
                            out=dnp[:],
                            in_=ex[:].rearrange("p g d h -> p g h d"),
                            axis=mybir.AxisListType.X,
                            op=mybir.AluOpType.add,
                        )
                        if side == 1:
                            nc.vector.tensor_tensor(
                                out=dn[:], in0=dn[:], in1=dnp[:], op=mybir.AluOpType.add
                            )
                        if stage == 3:
                            continue
                        exb = spool.tile([P, g, d, H], dt.bfloat16, tag=f"exb{side}")
                        nc.vector.tensor_copy(exb[:], ex[:])
                        mb = mbpool.tile([P, g * d, HF], dt.bfloat16, tag=f"mb{side}")
                        if os.environ.get("KNOMMUL"):
                            mbs.append(mb)
                            continue
                        nc.vector.tensor_tensor(
                            out=mb[:].rearrange("p s (h f) -> p s h f", f=F),
                            in0=gv[:, :, :, 0:HF].rearrange(
                                "p g d (h f) -> p (g d) h f", f=F
                            ),
                            in1=exb[:]
                            .rearrange("p g d h -> p (g d) h")
                            .to_broadcast([P, g * d, H, F]),
                            op=mybir.AluOpType.mult,
                        )
                        mbs.append(mb)

                    rcp = spool.tile([P, g, H], dt.float32, tag="rcp")
                    nc.vector.reciprocal(rcp[:], dn[:])
                    ot = opool.tile([P, g, HF], dt.float32, tag="ot")
                    # aggregate per block: identity-matmul over weighted rows
                    for gi in range(g if not os.environ.get("KNOMM") else 0):
                        ps_b = pspool.tile([P, HF], dt.float32, tag="ps")
                        nmm = dlo + dhi
                        i = 0
                        for mb, d in zip(mbs, (dlo, dhi)):
                            for k in range(d):
                                nc.tensor.matmul(
                                    ps_b[:],
                                    lhsT=id_sb[:],
                                    rhs=mb[:, gi * d + k, :],
                                    start=(i == 0),
                                    stop=(i == nmm - 1),
                                )
                                i += 1
                        nc.vector.tensor_tensor(
                            out=ot[:, gi, :].rearrange("p (h f) -> p h f", f=F),
                            in0=ps_b[:].rearrange("p (h f) -> p h f", f=F),
                            in1=rcp[:, gi, :].to_broadcast([P, H, F]),
                            op=mybir.AluOpType.mult,
                        )
                    nc.vector.tensor_tensor(
                        out=ot[:],
                        in0=ot[:],
                        in1=bias_sb[:].to_broadcast([P, g, HF]),
                        op=mybir.AluOpType.add,
                    )
                    nc.sync.dma_start(
                        out[bp0 * P : (bp0 + g) * P, :].rearrange(
                            "(g p) n -> p g n", p=P
                        ),
                        ot[:],
                    )


def kernel(feat, W, attn_l, attn_r, bias, src, dst):
    n = feat.shape[0]
    feat = np.asarray(feat, np.float32)
    W = np.asarray(W, np.float32)
    bias_np = np.asarray(bias, np.float32)
    cfg, per_core, (order, border) = host_prep(
        feat, W, np.asarray(attn_l, np.float32), np.asarray(attn_r, np.float32),
        bias_np, np.asarray(src), np.asarray(dst), n, hi_base=HI_BASE,
    )
    import os
    stage = int(os.environ.get("KSTAGE", "99"))
    import os as _os
    nc = bacc.Bacc("TRN2", target_bir_lowering=False, debug=False, num_devices=NCORES,
                   num_swdge_queues=int(_os.environ.get("KNQ", "4")))
    build_program(nc, cfg, stage=stage)
    res = run_bass_kernel_spmd(
        nc, per_core, core_ids=list(range(NCORES)), trace=TRACE
    )
    globals()["LAST_RESULTS"] = res
    outs = [res.results[c]["out"] for c in range(NCORES)]  # [nbc*128, 256] each

    full = np.zeros((cfg.npad, HF), np.float32)
    nb = cfg.npad // P
    for c in range(NCORES):
        o = outs[c].reshape(cfg.nbc, P, HF)
        js = border[np.arange(c, nb, NCORES)]  # global blocks owned by core c
        full[(js[:, None] * P + np.arange(P)).reshape(-1)] = o[: len(js)].reshape(
            -1, HF
        )
    # rank r holds node order[r]
    result = np.zeros((n, H, F), np.float32)
    result[order] = full[: n].reshape(n, H, F)
    return result

